# revision 11
# baseline (speedup 1.0000x reference)
"""Trainium2 Bass kernel for a 2-layer LSTM (T=512, B=64, IN=512, H=1024).

Sharding: tensor-parallel over the 4H gate dimension across 8 cores.
Each core owns 512 gate columns of every weight matrix, keeps h stationary
in the PE array (batch on the stationary axis), and streams the weight
columns (fp16, full PE rate).  Layer 0 runs on PE columns 0-63, layer 1 on
columns 64-127 (concurrent col-groups).  The layer-0 input projection is
computed inline on A-side slack.  One AllGather per timestep exchanges the
transposed h shards ([h0(s); h1(s-2)] — layer 1 lags by 2 slots so only 8
matmuls sit between the gather landing and the next gather firing).

Pointwise uses tanh only (sigmoid(x) = 0.5*tanh(x/2)+0.5) so the ACT
function table never reloads.  c-state stays fp32 and local; h is fp16.
"""

import numpy as np

T, B, IN, H = 512, 64, 512, 1024
NCORES = 8
HL = H // NCORES          # 128 h-columns per core
GL = 4 * HL               # 512 gate columns per core
KC_H = H // 128           # 8 contraction chunks over H

_CACHE = {}


def _build(T_steps):
    from concourse import bacc, tile, mybir, masks

    F16 = mybir.dt.float16
    F32 = mybir.dt.float32
    MUL = mybir.AluOpType.mult
    ADD = mybir.AluOpType.add
    TANH = mybir.ActivationFunctionType.Tanh

    nc = bacc.Bacc(
        "TRN2", target_bir_lowering=False, debug=False, num_devices=NCORES
    )

    # ---- kernel I/O ----
    xT = nc.dram_tensor("xT", [IN, T_steps, B], F16, kind="ExternalInput")
    wA = nc.dram_tensor("wA", [128, KC_H, GL], F16, kind="ExternalInput")
    wX = nc.dram_tensor("wX", [128, IN // 128, GL], F16, kind="ExternalInput")
    wB = nc.dram_tensor("wB", [128, 2 * KC_H, GL], F16, kind="ExternalInput")
    b0r = nc.dram_tensor("b0r", [1, GL], F16, kind="ExternalInput")
    b1r = nc.dram_tensor("b1r", [1, GL], F16, kind="ExternalInput")
    hab_init = nc.dram_tensor("hab_init", [128, NCORES * 128], F16, kind="ExternalInput")
    h1b_init = nc.dram_tensor("h1b_init", [128, B], F16, kind="ExternalInput")
    c_init = nc.dram_tensor("c_init", [128, HL], F32, kind="ExternalInput")

    ops_o = nc.dram_tensor("ops_o", [T_steps, B, HL], F16, kind="ExternalOutput")
    hT_o = nc.dram_tensor("hT_o", [2, B, HL], F16, kind="ExternalOutput")
    cT_o = nc.dram_tensor("cT_o", [2, B, HL], F32, kind="ExternalOutput")

    RG = [list(range(NCORES))]
    KX = IN // 128  # 4 x-projection chunks

    with tile.TileContext(nc) as tc:
        with (
            tc.tile_pool(name="const", bufs=1) as constp,
            tc.tile_pool(name="wts", bufs=1) as wts,
            tc.tile_pool(name="state", bufs=1) as state,
            tc.tile_pool(name="xs", bufs=3) as xsp,
            tc.tile_pool(name="pw", bufs=2) as pw,
            tc.tile_pool(name="pA", bufs=2, space="PSUM") as pAp,
            tc.tile_pool(name="pB", bufs=2, space="PSUM") as pBp,
            tc.tile_pool(name="pX", bufs=2, space="PSUM") as pXp,
            tc.tile_pool(name="pT", bufs=2, space="PSUM") as pTp,
            tc.tile_pool(name="ccin", bufs=3, space="DRAM") as ccinp,
            tc.tile_pool(name="ccout", bufs=3, space="DRAM") as ccoutp,
        ):
            # ---- persistent SBUF tiles ----
            ident = constp.tile([128, 128], F16)
            masks.make_identity(nc, ident[:])
            ones = constp.tile([1, 128], F16)
            nc.vector.memset(ones[:], 1.0)

            wh0_sb = wts.tile([128, KC_H, GL], F16)
            nc.sync.dma_start(wh0_sb[:], wA[:])
            wx_sb = wts.tile([128, KX, GL], F16)
            nc.sync.dma_start(wx_sb[:], wX[:])
            wB_sb = wts.tile([128, 2 * KC_H, GL], F16)
            nc.sync.dma_start(wB_sb[:], wB[:])
            b0_sb = wts.tile([1, GL], F16)
            nc.sync.dma_start(b0_sb[:], b0r[:])
            b1_sb = wts.tile([1, GL], F16)
            nc.sync.dma_start(b1_sb[:], b1r[:])

            # gathered h ring: hab[k][p, r, 0:64] = h0 chunk r (transposed),
            # hab[k][p, r, 64:128] = h1 chunk r.
            hab = [state.tile([128, NCORES, 128], F16, name=f"hab{i}") for i in range(3)]
            nc.sync.dma_start(hab[0][:], hab_init.ap().rearrange("p (r f) -> p r f", r=NCORES))

            c_tile = state.tile([128, HL], F32)
            nc.sync.dma_start(c_tile[:], c_init[:])
            h_tile = state.tile([128, HL], F16)
            bounce = state.tile([128, 128], F16)
            nc.sync.dma_start(bounce[:, 64:128], h1b_init[:, 0:64])
            nc.sync.dma_start(bounce[:, 0:64], h1b_init[:, 0:64])  # dummy init

            # x(s).T chunk ring (ring of 3 slots)
            xs = [xsp.tile([128, KX, B], F16, name=f"xs{i}", tag="xs") for i in range(3)]

            def load_x(t_idx, slot):
                # xT[:, t, :] -> [p, kc, b]
                src = xT[:, t_idx, :].rearrange("(kc p) b -> p kc b", p=128)
                nc.sync.dma_start(xs[slot][:], src)

            # prologue x loads: x(1), x(2)
            load_x(0, 1 % 3)
            if T_steps >= 2:
                load_x(1, 2 % 3)

            # prologue: xp(1) into a pX tile, then to SBUF (DVE can read only
            # one PSUM operand per op, so the gates-add needs xp in SBUF)
            pX = pXp.tile([128, GL], F32, tag="pX")
            for kc in range(KX):
                nc.tensor.matmul(
                    pX[0:B, :], xs[1 % 3][:, kc, 0:B], wx_sb[:, kc, :],
                    start=(kc == 0), stop=(kc == KX - 1),
                )
            xp_cur = pw.tile([B, GL], F32, tag="xps")
            nc.vector.tensor_copy(xp_cur[:], pX[0:B, :])

            # prologue: start pB for slot 3 (bias + Wi1 @ h0(1)) happens in slot 2.
            # pB ring handled inside the loop; pB_next started at slot s for slot s+1.
            pB_cur = None
            pB_next = None

            t0 = None  # persistent-ish pw scratch allocated per step from pool

            for s in range(1, T_steps + 3):
                doA = s <= T_steps              # L0 computes h0(s)
                doB = 3 <= s <= T_steps + 2     # L1 computes h1(s-2)
                startB = 2 <= s <= T_steps + 1  # start pB for slot s+1 (Wi1 @ h0(s-1))
                habA = hab[(s - 1) % 3]         # h0(s-1) [+ h1(s-3) in cols 64:]
                habB = hab[(s - 2) % 3]         # h0(s-2)

                # ---- A side: L0 recurrent gates (cols 0-63) ----
                if doA:
                    pA = pAp.tile([128, GL], F32, tag="pA")
                    nc.tensor.matmul(
                        pA[0:B, :], ones[0:1, 0:B], b0_sb[:], start=True, stop=False,
                    )
                    for kc in range(KC_H):
                        nc.tensor.matmul(
                            pA[0:B, :], habA[:, kc, 0:B], wh0_sb[:, kc, :],
                            start=False, stop=(kc == KC_H - 1),
                        )

                # ---- B side: finish L1 gates with Wh1 @ h1(s-3) (cols 64-127) ----
                if doB:
                    for kc in range(KC_H):
                        nc.tensor.matmul(
                            pB_cur[B : 2 * B, :], habA[:, kc, 64:128], wB_sb[:, KC_H + kc, :],
                            start=False, stop=(kc == KC_H - 1),
                            tile_position=(0, 64),
                        )

                # ---- pointwise L0 ----
                if doA:
                    t0 = pw.tile([B, GL], F32, tag="t0")
                    nc.vector.tensor_tensor(t0[:], pA[0:B, :], xp_cur[:], ADD)
                    tf = pw.tile([128, 2 * HL], F32, tag="tf")
                    tg = pw.tile([128, HL], F32, tag="tg")
                    to = pw.tile([128, HL], F32, tag="to")
                    nc.scalar.activation(tf[0:B, :], t0[:, 0 : 2 * HL], TANH, scale=0.5)
                    nc.scalar.activation(tg[0:B, :], t0[:, 2 * HL : 3 * HL], TANH)
                    nc.scalar.activation(to[0:B, :], t0[:, 3 * HL : 4 * HL], TANH, scale=0.5)

                # ---- pointwise L1 (reads psum directly) ----
                if doB:
                    nc.scalar.activation(
                        tf[B:128, :], pB_cur[B:128, 0 : 2 * HL], TANH, scale=0.5
                    )
                    nc.scalar.activation(tg[B:128, :], pB_cur[B:128, 2 * HL : 3 * HL], TANH)
                    nc.scalar.activation(
                        to[B:128, :], pB_cur[B:128, 3 * HL : 4 * HL], TANH, scale=0.5
                    )

                # ---- merged state update ----
                lo, hi = (0 if doA else B), (128 if doB else B)
                if lo < hi:
                    u = pw.tile([128, HL], F32, tag="u")
                    v = pw.tile([128, HL], F32, tag="v")
                    w = pw.tile([128, HL], F32, tag="w")
                    th = pw.tile([128, HL], F32, tag="th")
                    # u = (tanh(f/2)+1)*c ; v = (tanh(i/2)+1)*tanh(g) ; w = u+v = 2*c_new
                    nc.vector.scalar_tensor_tensor(
                        u[lo:hi, :], tf[lo:hi, HL : 2 * HL], 1.0, c_tile[lo:hi, :], ADD, MUL
                    )
                    nc.vector.scalar_tensor_tensor(
                        v[lo:hi, :], tf[lo:hi, 0:HL], 1.0, tg[lo:hi, :], ADD, MUL
                    )
                    nc.vector.tensor_tensor(w[lo:hi, :], u[lo:hi, :], v[lo:hi, :], ADD)
                    nc.vector.tensor_scalar_mul(c_tile[lo:hi, :], w[lo:hi, :], 0.5)
                    nc.scalar.activation(th[lo:hi, :], w[lo:hi, :], TANH, scale=0.5)
                    # h = 0.5*(tanh(o/2)+1)*tanh(c_new)
                    nc.vector.scalar_tensor_tensor(
                        h_tile[lo:hi, :], to[lo:hi, :], 1.0, th[lo:hi, :], ADD, MUL
                    )
                    nc.vector.tensor_scalar_mul(h_tile[lo:hi, :], h_tile[lo:hi, :], 0.5)

                # ---- transposes into bounce ----
                pT = pTp.tile([128, 128], F16, tag="pT")
                if doA:
                    nc.tensor.transpose(pT[:, 0:B], h_tile[0:B, :], ident[0:B, 0:B])
                    nc.vector.tensor_copy(bounce[:, 0:B], pT[:, 0:B])
                if doB:
                    nc.tensor.transpose(pT[:, B:128], h_tile[B:128, :], ident[B:128, B:128])
                    nc.vector.tensor_copy(bounce[:, B:128], pT[:, B:128])

                # ---- outputs ----
                if doB:
                    nc.sync.dma_start(ops_o[s - 3, :, :], h_tile[B:128, :])
                if s == T_steps:
                    nc.sync.dma_start(hT_o[0, :, :], h_tile[0:B, :])
                    nc.sync.dma_start(cT_o[0, :, :], c_tile[0:B, :])
                if s == T_steps + 2:
                    nc.sync.dma_start(hT_o[1, :, :], h_tile[B:128, :])
                    nc.sync.dma_start(cT_o[1, :, :], c_tile[B:128, :])

                # ---- allgather of [h0(s); h1(s-2)] ----
                if s <= T_steps + 1:
                    cc_in = ccinp.tile([128, 128], F16, tag="ccin")
                    cc_out = ccoutp.tile(
                        [NCORES * 128, 128], F16, tag="ccout", addr_space="Shared"
                    )
                    nc.sync.dma_start(cc_in[:], bounce[:])
                    nc.gpsimd.collective_compute(
                        "AllGather",
                        mybir.AluOpType.bypass,
                        replica_groups=RG,
                        ins=[cc_in.opt()],
                        outs=[cc_out.opt()],
                    )
                    dst = hab[s % 3]
                    src = cc_out.rearrange("(r p) f -> p r f", p=128)
                    half = NCORES // 2
                    nc.sync.dma_start(dst[:, 0:half, :], src[:, 0:half, :])
                    nc.sync.dma_start(dst[:, half:NCORES, :], src[:, half:NCORES, :])

                # ---- start next pB: bias + Wi1 @ h0(s-1) ----
                if startB:
                    pB_next = pBp.tile([128, GL], F32, tag="pB")
                    nc.tensor.matmul(
                        pB_next[B : 2 * B, :], ones[0:1, 64:128], b1_sb[:],
                        start=True, stop=False, tile_position=(0, 64),
                    )
                    for kc in range(KC_H):
                        nc.tensor.matmul(
                            pB_next[B : 2 * B, :], habA[:, kc, 0:B], wB_sb[:, kc, :],
                            start=False, stop=False, tile_position=(0, 64),
                        )
                    pB_cur = pB_next

                # ---- xp(s+1) on A-side slack ----
                if s <= T_steps - 1:
                    pX = pXp.tile([128, GL], F32, tag="pX")
                    for kc in range(KX):
                        nc.tensor.matmul(
                            pX[0:B, :], xs[(s + 1) % 3][:, kc, 0:B], wx_sb[:, kc, :],
                            start=(kc == 0), stop=(kc == KX - 1),
                        )
                    xp_next = pw.tile([B, GL], F32, tag="xps")
                    nc.vector.tensor_copy(xp_next[:], pX[0:B, :])
                    xp_cur = xp_next

                # ---- prefetch x(s+2) ----
                if s + 2 <= T_steps:
                    load_x(s + 1, (s + 2) % 3)

    nc.compile()
    return nc


def _pack_inputs(inputs, T_steps):
    x = np.asarray(inputs["x"])[:T_steps]
    h0 = np.asarray(inputs["h0"])
    c0 = np.asarray(inputs["c0"])
    W_ih0 = np.asarray(inputs["W_ih0"])
    W_hh0 = np.asarray(inputs["W_hh0"])
    W_ih1 = np.asarray(inputs["W_ih1"])
    W_hh1 = np.asarray(inputs["W_hh1"])
    b0 = np.asarray(inputs["b_ih0"]) + np.asarray(inputs["b_hh0"])
    b1 = np.asarray(inputs["b_ih1"]) + np.asarray(inputs["b_hh1"])

    f16 = np.float16
    xT_np = np.ascontiguousarray(x.transpose(2, 0, 1)).astype(f16)  # [IN, T, B]

    # hab_init[p, 128*r + f]: f<64 -> h0_init[f, 128r+p]; f>=64 -> h1_init[f-64, ...]
    hab0 = np.zeros((128, NCORES * 128), f16)
    for r in range(NCORES):
        hab0[:, 128 * r : 128 * r + B] = h0[0][:, 128 * r : 128 * (r + 1)].T
        hab0[:, 128 * r + B : 128 * (r + 1)] = h0[1][:, 128 * r : 128 * (r + 1)].T

    in_maps = []
    for r in range(NCORES):
        # core r owns the r-th 128-row block of each of the 4 gates (i,f,g,o)
        sl = np.concatenate(
            [np.arange(g * H + r * HL, g * H + (r + 1) * HL) for g in range(4)]
        )
        hsl = slice(r * HL, (r + 1) * HL)

        def streams(Wslice):  # [GL, K] -> [128, K//128, GL]
            K = Wslice.shape[1]
            return np.ascontiguousarray(
                Wslice.T.reshape(K // 128, 128, GL).transpose(1, 0, 2)
            ).astype(f16)

        wA_np = streams(W_hh0[sl])
        wX_np = streams(W_ih0[sl])
        wB_np = np.concatenate([streams(W_ih1[sl]), streams(W_hh1[sl])], axis=1)

        cini = np.zeros((128, HL), np.float32)
        cini[0:B] = c0[0][:, hsl]
        cini[B:128] = c0[1][:, hsl]

        in_maps.append(
            {
                "xT": xT_np,
                "wA": wA_np,
                "wX": wX_np,
                "wB": wB_np,
                "b0r": b0[sl][None, :].astype(f16),
                "b1r": b1[sl][None, :].astype(f16),
                "hab_init": hab0,
                "h1b_init": np.ascontiguousarray(h0[1][:, hsl].T).astype(f16),
                "c_init": cini,
            }
        )
    return in_maps


def _unpack(results, T_steps):
    ops = np.zeros((T_steps, B, H), np.float32)
    hT = np.zeros((2, B, H), np.float32)
    cT = np.zeros((2, B, H), np.float32)
    for r in range(NCORES):
        hsl = slice(r * HL, (r + 1) * HL)
        ops[:, :, hsl] = results[r]["ops_o"].astype(np.float32)
        hT[:, :, hsl] = results[r]["hT_o"].astype(np.float32)
        cT[:, :, hsl] = results[r]["cT_o"].astype(np.float32)
    return ops, hT, cT


def _install_ntff_hook():
    """Reconstruct the antenv.axon_hooks shim so trace=True can reach the
    NTFF profiling ABI in libaxon_pjrt.so (the agent image lacks the shim)."""
    import sys
    import types

    try:
        from antenv import axon_hooks  # noqa: F401

        return
    except ImportError:
        pass
    import antenv
    from trn_agent_boot.trn_boot import _ntff_profile_via_ctypes

    mod = types.ModuleType("antenv.axon_hooks")
    holder = {}
    mod.set_axon_ntff_profile_hook = lambda h: holder.__setitem__("h", h)
    mod.get_axon_ntff_profile_hook = lambda: holder.get("h")
    sys.modules["antenv.axon_hooks"] = mod
    antenv.axon_hooks = mod
    hook = _ntff_profile_via_ctypes("/opt/axon/libaxon_pjrt.so")
    if hook is not None:
        mod.set_axon_ntff_profile_hook(hook)

    # avoid the S3 artifact upload inside the profile path
    from concourse import bass_utils

    bass_utils.upload_artifacts = lambda tmpdir: tmpdir


def run_on_hw(inputs, T_steps=T, trace=False):
    from concourse.bass_utils import run_bass_kernel_spmd

    if trace:
        try:
            _install_ntff_hook()
        except Exception as e:  # pragma: no cover
            print(f"ntff hook install failed: {e}")

    if T_steps not in _CACHE:
        _CACHE[T_steps] = _build(T_steps)
    nc = _CACHE[T_steps]
    in_maps = _pack_inputs(inputs, T_steps)
    res = run_bass_kernel_spmd(nc, in_maps, list(range(NCORES)), trace=trace)
    return _unpack(res.results, T_steps), res


def kernel(**inputs):
    (ops, hT, cT), _ = run_on_hw(inputs, T)
    return ops, hT, cT
